# revision 35
# baseline (speedup 1.0000x reference)
"""GCN aggregator kernel for Trainium2 (8 NeuronCores, batch-sharded).

Math (faithful to the reference):
    mask[n, c] = 1 iff c in set(neigh_idx[n, :]) | {nodes[n]}     (N x M 0/1)
    out = diag(1/sqrt(row_sum)) @ mask @ diag(1/sqrt(max(col_sum,1))) @ E

All mask normalisation is a pure function of the index inputs, so the host
precomputes it exactly (set-semantics dedup flags w, global column counts
via bincount, row counts):
    beta[n, k] = w[n, k] / sqrt(max(colcount[idx[n, k]], 1))
    rowinv[n]  = 1 / sqrt(sum_k w[n, k])
    out[n, :]  = rowinv[n] * sum_k beta[n, k] * E[idx[n, k], :]

The device kernel is the memory-bound core: a 16896-row gather of the
(bf16) embedding table and the weighted per-row reduction.

Per-core (512 rows) layout: entry i = g*128 + p with g = nb*33 + k, so
gathered row i lands at [partition p, slot g] = row nb*128+p's k-th
neighbour.  Device algorithm:
  1. G[p, g, :] = E16[idx, :] via chunked dma_gather (2048-idx chunks,
     two in flight in the SWDGE ring).
  2. While the gather streams: DG[:, g, :] = diag(beta[:, g]) built with
     one tensor_scalar per slot (identity * per-partition f32 scalar).
  3. PE accumulates out_block = sum_k diag(beta_k) @ G_k per 128-row
     block in PSUM, trailing the gather chunk by chunk.
  4. PSUM -> SBUF copy on the Activation engine applies the rowinv scale;
     per-block DMA stores.
"""

import numpy as np

N, K, M, D = 4096, 32, 16384, 128
NCORES = 8
NPR = N // NCORES  # 512 rows per core
KP1 = K + 1  # 33 entries per row
P = 128
NB = NPR // P  # 4 row-blocks per core
GW = NB * KP1  # 132 entries per partition
NI = P * GW  # 16896 entries per core
MAIN_CH = 1024  # main-gather chunk (descs; >1024 fails on the NRT)

_NC_CACHE = {}


def _apply_tile_patches():
    """Work around this walrus build's 1-embedded-sync-wait-per-instruction
    limit: split the kernel-tail drain (the one place Tile emits a
    multi-wait instruction unconditionally) into a chain of single-wait
    drains. SP is in-order, so this is equivalent."""
    import concourse.mybir as mybir
    import concourse.tile as tile
    import concourse.tile_sem_assignment as tsa

    # Cap the DMA completion-sem lanes so the drain chain stays short.
    tsa.NUM_SWDGE_GLOBAL_SEMS = 6

    if getattr(tile.TileContext, "_split_drain_patch", False):
        return
    from concourse.vector_clock import ScopedClock

    def _drain_and_barrier(self, tick_clock, wait_clock):
        probe = self.nc.sync.drain()
        wait_clock.add_sem_waits(
            probe.ins, ScopedClock({None: tick_clock.global_clock})
        )
        si = probe.ins.sync_info
        waits = list(si.on_wait) if si is not None else []
        if len(waits) > 1:
            si.on_wait = waits[:1]
            for w in waits[1:]:
                d = self.nc.sync.drain()
                dsi = d.ins.sync_info
                if dsi is None:
                    d.ins.sync_info = mybir.SyncInfo(on_wait=[w], on_update=[])
                else:
                    dsi.on_wait = [w]
        self.nc.all_engine_barrier()
        assert self.sems is not None
        popped = self.nc._tile_sem_poison_stack.pop()
        assert popped is self._sem_poison
        self.nc.clear_and_free_semaphores(list(self.sems.allocated().values()))
        self.nc.all_engine_barrier()

    tile.TileContext._drain_and_barrier = _drain_and_barrier
    tile.TileContext._split_drain_patch = True


def _chunked_gather(nc, out_view, src_ap, idx_tile, elem, total, base=0,
                    ch_max=MAIN_CH, sizes=None):
    """Issue dma_gather chunks over [base, base+total).  `sizes` gives an
    explicit chunk-size schedule (e.g. small lead chunk so the pipeline
    starts early, small tail chunks so the last consumer chain is short);
    remaining space is covered by ch_max chunks."""
    plan = []
    left = total
    for s in (sizes or []):
        plan.append(s)
        left -= s
    n_full = left // ch_max
    plan[1:1] = [ch_max] * n_full
    if left - n_full * ch_max:
        plan.insert(1 + n_full, left - n_full * ch_max)
    pos = 0
    for ch in plan:
        nc.gpsimd.dma_gather(
            out_view[:, pos // P : (pos + ch) // P, :],
            src_ap,
            idx_tile[:, (base + pos) // 16 : (base + pos + ch) // 16],
            ch,
            ch,
            elem,
        )
        pos += ch
    assert pos == total


def _build_nc(reps=1, ablate=()):
    import concourse.bacc as bacc
    import concourse.mybir as mybir
    import concourse.tile as tile
    from contextlib import ExitStack

    _apply_tile_patches()

    dt = mybir.dt
    Alu = mybir.AluOpType
    Act = mybir.ActivationFunctionType

    nc = bacc.Bacc(
        "TRN2",
        target_bir_lowering=False,
        debug=False,
        num_devices=NCORES,
        dynamic_dma_scratch_size=1 << 15,
    )

    bt_d = nc.dram_tensor("beta", [P, GW], dt.float32, kind="ExternalInput")
    idxw_d = nc.dram_tensor("idxw", [P, NI // 16], dt.int16, kind="ExternalInput")
    emb_d = nc.dram_tensor("embed", [M, D], dt.bfloat16, kind="ExternalInput")
    out_d = nc.dram_tensor("out", [NPR, D], dt.float32, kind="ExternalOutput")

    with tile.TileContext(nc) as tc, ExitStack() as ctx:
        sb = ctx.enter_context(tc.tile_pool(name="sb", bufs=1))
        ps2 = ctx.enter_context(tc.tile_pool(name="ps2", bufs=2, space="PSUM"))

        def _body():
         # ---- iota first (Pool queue is in-order; don't park it behind the
         # gather dispatches), then the index load the gather needs (in two
         # halves so the first dispatch starts sooner)
         # index load on the (otherwise idle) SP queue, first chunk's worth
         # first so the lead gather dispatch isn't gated on the full load
         IW = sb.tile([P, NI // 16], dt.int16)
         nc.sync.dma_start(out=IW[:, 0:64], in_=idxw_d.ap()[:, 0:64])
         nc.sync.dma_start(out=IW[:, 64:], in_=idxw_d.ap()[:, 64:])

         # ---- main gather (overlaps everything below)
         G = sb.tile([P, GW, D], dt.bfloat16)
         if "nogather" in ablate:
             nc.vector.memset(G[:, 0:1, :], 1.0)
         else:
             _chunked_gather(nc, G[:], emb_d.ap(), IW[:], D, NI)

         # ---- iotas after the gather dispatches (Pool queue is in-order;
         # the DVE consumers have plenty of slack)
         iot0 = sb.tile([P, P], dt.int16)
         nc.gpsimd.iota(iot0[:], pattern=[[1, P]], base=0, channel_multiplier=0)
         pid0 = sb.tile([P, 1], dt.int16)
         nc.gpsimd.iota(pid0[:], pattern=[[0, 1]], base=0, channel_multiplier=1)

         # ---- small input loads (Activation queue: idle)
         bt = sb.tile([P, GW], dt.float32)
         nc.scalar.dma_start(out=bt[:], in_=bt_d.ap())

         # ---- identity matrix (bf16)
         iot = sb.tile([P, P], dt.int16)
         nc.vector.tensor_copy(out=iot[:], in_=iot0[:])
         pid = sb.tile([P, 1], dt.int16)
         nc.vector.tensor_copy(out=pid[:], in_=pid0[:])
         IDN = sb.tile([P, P], dt.bfloat16)
         nc.vector.tensor_tensor(
             out=IDN[:], in0=iot[:], in1=pid[:].to_broadcast([P, P]),
             op=Alu.is_equal,
         )

         # ---- all 132 diag(beta) matrices up front (independent of G)
         DG = sb.tile([P, GW, P], dt.bfloat16)
         for g in range(GW):
             nc.vector.tensor_scalar(
                 out=DG[:, g, :], in0=IDN[:], scalar1=bt[:, g : g + 1],
                 scalar2=None, op0=Alu.mult,
             )

         # ---- weighted sum on PE, trailing the gather chunk by chunk.
         # rowinv is folded into beta on the host, so the PSUM block is the
         # final output: copy to SBUF and store per block (SP queue).
         osb = sb.tile([P, NB, D], dt.float32)
         for nb in range(NB):
             ops_ = ps2.tile([P, D], dt.float32, tag="opsblk")
             if "notail" in ablate:
                 nc.vector.memset(osb[:, nb, :], 0.0)
             else:
                 for k in range(KP1):
                     g = nb * KP1 + k
                     nc.tensor.matmul(
                         out=ops_[:],
                         lhsT=DG[:, g, :],
                         rhs=G[:, g, :],
                         start=(k == 0),
                         stop=(k == KP1 - 1),
                     )
                 nc.vector.tensor_copy(out=osb[:, nb, :], in_=ops_[:])
             nc.sync.dma_start(
                 out=out_d.ap().rearrange("(nb p) d -> p nb d", p=P)[
                     :, nb : nb + 1, :
                 ],
                 in_=osb[:, nb : nb + 1, :],
             )


        # repeated body for differential wall-clock timing
        for _rep in range(reps):
            _body()

    nc.compile()
    return nc


def get_nc(reps=1, ablate=()):
    key = ("nc", reps, tuple(ablate))
    if key not in _NC_CACHE:
        _NC_CACHE[key] = _build_nc(reps, tuple(ablate))
    return _NC_CACHE[key]


def _wrap16(entries):
    """entries: [NI] int -> int16 wrapped layout [128, NI//16]: entry i at
    partition i%16, column i//16, replicated across the 8 groups."""
    s = entries.reshape(-1, 16).T.astype(np.int16)  # [16, NI//16]
    return np.ascontiguousarray(np.tile(s, (8, 1)))


def prep_inputs(nodes, neigh_idx, embed_matrix):
    import ml_dtypes

    nodes = np.asarray(nodes)
    neigh_idx = np.asarray(neigh_idx)
    emb16 = np.ascontiguousarray(
        np.asarray(embed_matrix, dtype=np.float32).astype(ml_dtypes.bfloat16)
    )

    idx_full = np.concatenate([neigh_idx, nodes[:, None]], axis=1).astype(
        np.int32
    )  # [N, 33]
    # first-occurrence flags (set semantics): w[n,k] = 1 iff no j<k with
    # idx[n,j] == idx[n,k]
    eq = idx_full[:, None, :] == idx_full[:, :, None]  # [N, 33, 33]
    dupcnt = np.tril(eq, -1).sum(axis=2)  # [N, 33] prior occurrences
    w = (dupcnt == 0)
    # exact global column counts over the deduped mask
    colcnt = np.bincount(idx_full[w].ravel(), minlength=M).astype(np.float64)
    col_inv = (1.0 / np.sqrt(np.maximum(colcnt, 1.0))).astype(np.float32)
    beta = np.where(w, col_inv[idx_full], np.float32(0.0))
    rowinv = 1.0 / np.sqrt(w.sum(axis=1))  # [N]
    beta = (beta * rowinv[:, None]).astype(np.float32)

    def core_layout(x):  # [NPR, KP1] -> [P, GW] with col g = (nb, k)
        return np.ascontiguousarray(
            x.reshape(NB, P, KP1).transpose(1, 0, 2).reshape(P, GW)
        )

    in_maps = []
    for c in range(NCORES):
        sl = slice(c * NPR, (c + 1) * NPR)
        slab = idx_full[sl]  # [512, 33]
        # entry order i = g*128 + p, g = nb*33 + k  ->  value idx[nb*128+p, k]
        e = slab.reshape(NB, P, KP1).transpose(0, 2, 1).reshape(NI)
        in_maps.append(
            {
                "beta": core_layout(beta[sl]),
                "idxw": _wrap16(e),
                "embed": emb16,
            }
        )
    return in_maps


def kernel(nodes, neigh_idx, embed_matrix):
    nc = get_nc()
    from concourse.bass_utils import run_bass_kernel_spmd

    in_maps = prep_inputs(nodes, neigh_idx, embed_matrix)
    res = run_bass_kernel_spmd(nc, in_maps, core_ids=list(range(NCORES)))
    out = np.concatenate([res.results[c]["out"] for c in range(NCORES)], axis=0)
    return out.astype(np.float32)
